# revision 12
# baseline (speedup 1.0000x reference)
"""Capsule-routing kernel for 8 Trainium2 NeuronCores.

Problem: u_hat = einsum('nidk,bik->bnid', W, x); 3 rounds of dynamic
routing (softmax over n, weighted sum over i, squash, agreement update).

Sharding: input-capsule axis i (2048) split 8 ways -> 256 i per core.
Softmax over n is local; the per-iteration weighted sum s[b,n,d] is a
partial over local i, combined with an on-device AllReduce (iterations
1,2) or on the host (final iteration).

Per-core schedule (B=32, N=64, IL=256, D=32, K=16):
  sweep 1: u_hat via TensorE (block-diag x lhsT, K=64, M=128 -> psum
           [(j,b), (d,n)]), drain-cast to fp16, store to DRAM; S0
           accumulated in psum via ones-matmul. AllReduce S0, squash
           -> out0 [128, 2048] fp16 (partition-replicated x4).
  sweep 2/3 (per pair of 4-i groups): load u16 [128,4096], DVE
           tmp=u16*out_rep, halving-tree over d -> agreement a,
           b_state update, exp (ACT), Z (DVE reduce over n free dim),
           c=e/Z (ACT per-partition scale), c_rep half-depth doubling
           (ACT), sm=u16*c_rep in 2 d-chunks (DVE), ones-matmul
           accumulates s partial in psum (PE). AllReduce+squash between
           sweeps; final partial summed+squashed on host.

Layouts: u16 partition p = 32*j + b (j = i mod 4 within group), free
(d,n) d-major so d-reductions/broadcasts are contiguous halving trees.
"""
import sys
import types

sys.path.insert(0, "/opt/trn_rl_repo")

import numpy as np

from concourse import bacc, tile, mybir
from concourse.bass_utils import run_bass_kernel_spmd

f32 = mybir.dt.float32
f16 = mybir.dt.float16
AX = mybir.AxisListType
OP = mybir.AluOpType
AF = mybir.ActivationFunctionType

B, N, I, D, K = 32, 64, 2048, 32, 16
NCORES = 8
IL = I // NCORES          # 256 local input capsules
G = IL // 4               # 64 groups of 4 i
NP = G // 2               # 32 group-pairs
DN = D * N                # 2048 free elements per group, d-major
INV_LOG2 = float(1.0 / np.log(2.0))


def _install_ntff_hook():
    if "antenv.axon_hooks" in sys.modules:
        return
    try:
        mod = types.ModuleType("antenv.axon_hooks")
        state = {"hook": None}
        mod.set_axon_ntff_profile_hook = lambda h: state.__setitem__("hook", h)
        mod.get_axon_ntff_profile_hook = lambda: state["hook"]
        sys.modules["antenv.axon_hooks"] = mod
        import antenv
        antenv.axon_hooks = mod
        from trn_agent_boot.trn_boot import _ntff_profile_via_ctypes
        mod.set_axon_ntff_profile_hook(
            _ntff_profile_via_ctypes("/opt/axon/libaxon_pjrt.so"))
    except Exception:
        pass


def _build():
    nc = bacc.Bacc("TRN2", target_bir_lowering=False, debug=False,
                   num_devices=NCORES)

    w_t2 = nc.dram_tensor("w_t2", [NP, 128, DN], f16, kind="ExternalInput")
    x_bd = nc.dram_tensor("x_bd", [128, NP, 128], f16, kind="ExternalInput")
    s2_part = nc.dram_tensor("s2_part", [B, DN], f32, kind="ExternalOutput")

    u_store = nc.dram_tensor("u_store", [G, 128, DN], f16)
    cc_in = [nc.dram_tensor(f"cc_in{r}", [B, DN], f32) for r in range(2)]
    cc_out = [nc.dram_tensor(f"cc_out{r}", [B, DN], f32, addr_space="Shared")
              for r in range(2)]

    ones4_np = np.zeros((128, 32), np.float16)
    for p in range(128):
        ones4_np[p, p % 32] = 1.0
    ones4 = nc.inline_tensor(ones4_np, name="ones4")
    ones4f = nc.inline_tensor(ones4_np.astype(np.float32), name="ones4f")

    core_ids = list(range(NCORES))

    with tile.TileContext(nc) as tc:
        with tc.tile_pool(name="const", bufs=1) as constp, \
             tc.tile_pool(name="tail", bufs=1) as tail, \
             tc.tile_pool(name="small", bufs=4) as small, \
             tc.tile_pool(name="bstate", bufs=1) as bstate, \
             tc.tile_pool(name="psacc", bufs=1, space="PSUM") as psacc:

            ones_sb = constp.tile([128, 32], f16)
            nc.sync.dma_start(ones_sb[:], ones4[:])
            ones_sbf = constp.tile([128, 32], f32)
            nc.sync.dma_start(ones_sbf[:], ones4f[:])
            out_rep = [constp.tile([128, DN], f16, tag=f"orep{r}",
                                   name=f"orep{r}") for r in range(2)]

            def squash_to_outrep(s_sb, orep, pre_scale):
                """orep [128, (d,n)] f16 <- x4-replicated squash(s_sb*pre_scale)."""
                ps2 = float(pre_scale * pre_scale)
                sq = tail.tile([32, D, N], f32, tag="t_sq")
                nc.scalar.square(sq[:],
                                 s_sb[:].rearrange("p (d n) -> p d n", n=N))
                cur, d = sq, D
                while d > 2:
                    nxt = tail.tile([32, d // 2, N], f32, tag=f"t_tr{d}")
                    nc.vector.tensor_add(nxt[:], cur[:, 0:d // 2, :],
                                         cur[:, d // 2:d, :])
                    cur, d = nxt, d // 2
                sn = tail.tile([32, 1, N], f32, tag="t_sn")
                nc.vector.tensor_add(sn[:], cur[:, 0:1, :], cur[:, 1:2, :])
                r_ = tail.tile([32, N], f32, tag="t_r")
                nc.scalar.activation(r_[:], sn[:, 0, :], AF.Sqrt,
                                     bias=0.0, scale=ps2)
                den = tail.tile([32, N], f32, tag="t_den")
                nc.vector.tensor_scalar(den[:], sn[:, 0, :], ps2, 1.0,
                                        OP.mult, OP.add)
                rd = tail.tile([32, N], f32, tag="t_rd")
                nc.vector.reciprocal(rd[:], den[:])
                fac = tail.tile([32, N], f32, tag="t_fac")
                nc.vector.scalar_tensor_tensor(fac[:], r_[:],
                                               float(pre_scale), rd[:],
                                               op0=OP.mult, op1=OP.mult)
                frep = tail.tile([32, D, N], f32, tag="t_frep")
                nc.scalar.copy(frep[:, 0:1, :], fac[:].unsqueeze(1))
                d = 1
                while d < D:
                    nc.scalar.copy(frep[:, d:2 * d, :], frep[:, 0:d, :])
                    d *= 2
                o16 = tail.tile([32, D, N], f16, tag="t_o16")
                nc.vector.tensor_mul(
                    o16[:], s_sb[:].rearrange("p (d n) -> p d n", n=N), frep[:])
                for j in range(4):
                    nc.sync.dma_start(
                        orep[32 * j:32 * j + 32, :],
                        o16[:].rearrange("p d n -> p (d n)"))

            # ---------------- sweep 1: u_hat + S0 ----------------
            s0_ps = psacc.tile([B, DN], f32, tag="sacc")
            with tc.tile_pool(name="xw", bufs=1) as xw, \
                 tc.tile_pool(name="wp", bufs=3) as wp, \
                 tc.tile_pool(name="u16s1", bufs=3) as u16s1, \
                 tc.tile_pool(name="psum1", bufs=2, space="PSUM") as psum1:
                xbd_sb = xw.tile([128, NP, 128], f16)
                nc.sync.dma_start(xbd_sb[:], x_bd[:])
                s0_sb = xw.tile([128, DN], f32)
                for gp in range(NP):
                    wt = wp.tile([128, DN], f16)
                    nc.sync.dma_start(wt[:], w_t2[gp])
                    for gs in range(2):
                        g = 2 * gp + gs
                        u16 = u16s1.tile([128, DN], f16)
                        for h in range(2):
                            pu = psum1.tile([128, DN // 2], f32)
                            for ch in range(2):
                                nc.tensor.matmul(
                                    pu[:, 512 * ch:512 * (ch + 1)],
                                    lhsT=xbd_sb[64 * gs:64 * (gs + 1), gp, :],
                                    rhs=wt[64 * gs:64 * (gs + 1),
                                           1024 * h + 512 * ch:
                                           1024 * h + 512 * (ch + 1)],
                                    start=True, stop=True)
                            nc.scalar.copy(
                                u16[:, 1024 * h:1024 * (h + 1)], pu[:])
                            sl = s0_sb[:, 1024 * h:1024 * (h + 1)]
                            if gp == 0 and gs == 0:
                                nc.vector.tensor_copy(sl, pu[:])
                            else:
                                nc.vector.tensor_add(sl, sl, pu[:])
                        nc.sync.dma_start(u_store[g], u16[:])

                # fold j-slots: s0_ps[32, DN] = ones4f.T @ s0_sb
                for ch in range(4):
                    nc.tensor.matmul(
                        s0_ps[:, 512 * ch:512 * (ch + 1)],
                        lhsT=ones_sbf[:],
                        rhs=s0_sb[:, 512 * ch:512 * (ch + 1)],
                        start=True, stop=True)
                # S0 allreduce + squash -> out_rep[0]
                s0_dr = tail.tile([B, DN], f32, tag="t_drain")
                nc.scalar.copy(s0_dr[:], s0_ps[:])
                nc.sync.dma_start(cc_in[0][:], s0_dr[:])
                nc.gpsimd.collective_compute(
                    "AllReduce", OP.add, ins=[cc_in[0][:]],
                    outs=[cc_out[0][:]], replica_groups=[core_ids])
                s0_all = tail.tile([B, DN], f32, tag="t_all")
                nc.sync.dma_start(s0_all[:], cc_out[0][:])
                squash_to_outrep(s0_all, out_rep[0], 1.0 / 64.0)

            # ---------------- sweeps 2 and 3: routing ----------------
            with tc.tile_pool(name="u16p", bufs=5) as u16p, \
                 tc.tile_pool(name="crepp", bufs=4) as crepp, \
                 tc.tile_pool(name="big", bufs=4) as big, \
                 tc.tile_pool(name="tree", bufs=3) as tree:
                bs_tiles = []
                for it in range(2):
                    s_ps = psacc.tile([B, DN], f32, tag="sacc")
                    first_mm = True
                    for gp in range(NP):
                        u16 = u16p.tile([128, 2, DN], f16)
                        nc.sync.dma_start(
                            u16[:],
                            u_store[2 * gp:2 * gp + 2].transpose([1, 0, 2]))
                        u4 = u16[:].rearrange("p a (d n) -> p a d n", n=N)
                        orep4 = (out_rep[it][:]
                                 .rearrange("p (d n) -> p d n", n=N)
                                 .unsqueeze(1).broadcast_to([128, 2, D, N]))
                        tmp = big.tile([128, 2, D, N], f16, tag="tmp")
                        nc.vector.tensor_mul(tmp[:], u4, orep4)
                        cur, d = tmp, D
                        while d > 2:
                            nxt = tree.tile([128, 2, d // 2, N], f16,
                                            tag=f"tr{d}")
                            nc.vector.tensor_add(nxt[:], cur[:, :, 0:d // 2, :],
                                                 cur[:, :, d // 2:d, :])
                            cur, d = nxt, d // 2
                        if it == 0:
                            bs = bstate.tile([128, 2, 1, N], f32,
                                             tag=f"bs{gp}", name=f"bs{gp}")
                            bs_tiles.append(bs)
                            nc.vector.tensor_add(bs[:], cur[:, :, 0:1, :],
                                                 cur[:, :, 1:2, :])
                        else:
                            bs = bs_tiles[gp]
                            a2 = small.tile([128, 2, 1, N], f32, tag="a2")
                            nc.vector.tensor_add(a2[:], cur[:, :, 0:1, :],
                                                 cur[:, :, 1:2, :])
                            nc.vector.tensor_add(bs[:], bs[:], a2[:])
                        m2 = small.tile([128, 2, 1], f32, tag="m2")
                        nc.vector.tensor_reduce(out=m2[:], in_=bs[:, :, 0, :],
                                                axis=AX.X, op=OP.max)
                        nm2 = small.tile([128, 2, 1], f32, tag="nm2")
                        nc.vector.tensor_scalar_mul(nm2[:], m2[:], -INV_LOG2)
                        e2 = small.tile([128, 2, N], f16, tag="e2")
                        for gs in range(2):
                            nc.scalar.activation(e2[:, gs, :], bs[:, gs, 0, :],
                                                 AF.Exp, bias=nm2[:, gs, :],
                                                 scale=INV_LOG2)
                        z2 = small.tile([128, 2, 1], f32, tag="z2")
                        nc.vector.tensor_reduce(out=z2[:], in_=e2[:],
                                                axis=AX.X, op=OP.add)
                        rz = small.tile([128, 2, 1], f32, tag="rz")
                        nc.vector.reciprocal(rz[:], z2[:])
                        cz4 = small.tile([128, 2, 32], f16, tag="cz4")
                        for gs in range(2):
                            nc.vector.tensor_mul(
                                cz4[:, gs, :], ones_sb[:],
                                rz[:, gs, :].broadcast_to([128, 32]))
                        crep = crepp.tile([128, 2, D // 4, N], f16, tag="crep")
                        nc.scalar.copy(crep[:, :, 0:1, :], e2[:].unsqueeze(2))
                        d = 1
                        while d < D // 4:
                            nc.scalar.copy(crep[:, :, d:2 * d, :],
                                           crep[:, :, 0:d, :])
                            d *= 2
                        sm = big.tile([128, 2, D, N], f16, tag="sm")
                        for hd in range(4):
                            nc.vector.tensor_mul(
                                sm[:, :, 8 * hd:8 * (hd + 1), :],
                                u4[:, :, 8 * hd:8 * (hd + 1), :], crep[:])
                        smf = sm[:].rearrange("p a d n -> p a (d n)")
                        for gs in range(2):
                            for ch in range(4):
                                nc.tensor.matmul(
                                    s_ps[:, 512 * ch:512 * (ch + 1)],
                                    lhsT=cz4[:, gs, :],
                                    rhs=smf[:, gs, 512 * ch:512 * (ch + 1)],
                                    start=first_mm,
                                    stop=(gp == NP - 1 and gs == 1),
                                    skip_group_check=True)
                            first_mm = False

                    s_sb = tail.tile([B, DN], f32, tag="t_drain")
                    nc.scalar.copy(s_sb[:], s_ps[:])
                    if it == 0:
                        nc.sync.dma_start(cc_in[1][:], s_sb[:])
                        nc.gpsimd.collective_compute(
                            "AllReduce", OP.add, ins=[cc_in[1][:]],
                            outs=[cc_out[1][:]], replica_groups=[core_ids])
                        s_all = tail.tile([B, DN], f32, tag="t_all")
                        nc.sync.dma_start(s_all[:], cc_out[1][:])
                        squash_to_outrep(s_all, out_rep[1], 1.0)
                    else:
                        nc.sync.dma_start(s2_part[:], s_sb[:])

    nc.compile()
    return nc


_NC_CACHE = {}


def _get_nc():
    if "nc" not in _NC_CACHE:
        _NC_CACHE["nc"] = _build()
    return _NC_CACHE["nc"]


def _prep_core(x_c, w_c):
    """x_c [B, IL, K] f32, w_c [N, IL, D, K] f32 -> in_map dict."""
    wt = np.ascontiguousarray(w_c.transpose(1, 3, 2, 0))  # [IL, K, D, N]
    wt2 = wt.reshape(NP, 8, K, DN).reshape(NP, 128, DN).astype(np.float16)
    xt = x_c.transpose(1, 2, 0)  # [IL, K, B]
    x_bd = np.zeros((128, NP, 128), np.float16)
    for g in range(G):
        q, s = g // 2, g % 2
        for j in range(4):
            i = 4 * g + j
            x_bd[s * 64 + j * 16:s * 64 + j * 16 + K, q,
                 j * 32:j * 32 + 32] = xt[i].astype(np.float16)
    return {"w_t2": wt2, "x_bd": x_bd}


def _squash_np(v):
    sn = np.sum(v * v, axis=-1, keepdims=True)
    return np.sqrt(sn) / (1.0 + sn) * v


def _run(inputs, W, trace=False):
    _install_ntff_hook()
    nc = _get_nc()
    x = np.asarray(inputs, np.float32)
    Wf = np.asarray(W, np.float32)
    in_maps = []
    for c in range(NCORES):
        sl = slice(c * IL, (c + 1) * IL)
        in_maps.append(_prep_core(x[:, sl, :], Wf[:, sl, :, :]))
    res = run_bass_kernel_spmd(nc, in_maps, list(range(NCORES)), trace=trace)
    s2 = np.zeros((B, DN), np.float64)
    for c in range(NCORES):
        s2 += res.results[c]["s2_part"].astype(np.float64)
    s2 = s2.reshape(B, D, N).transpose(0, 2, 1).astype(np.float32)
    out = _squash_np(s2).astype(np.float32)
    return out, res


def kernel(inputs, W):
    out, _ = _run(inputs, W, trace=False)
    return out


# revision 15
# speedup vs baseline: 1.0259x; 1.0259x over previous
"""Capsule-routing kernel for 8 Trainium2 NeuronCores.

Problem: u_hat = einsum('nidk,bik->bnid', W, x); 3 rounds of dynamic
routing (softmax over n, weighted sum over i, squash, agreement update).

Sharding: input-capsule axis i (2048) split 8 ways -> 256 i per core.
Softmax over n is local; the per-iteration weighted sum s[b,n,d] is a
partial over local i, combined with an on-device AllReduce (iterations
1,2) or on the host (final iteration).

Per-core schedule (B=32, N=64, IL=256, D=32, K=16):
  sweep 1: u_hat via TensorE (block-diag x lhsT, K=64, M=128 -> psum
           [(j,b), (d,n)]), drain-cast to fp16, store to DRAM; S0
           accumulated in psum via ones-matmul. AllReduce S0, squash
           -> out0 [128, 2048] fp16 (partition-replicated x4).
  sweep 2/3 (per pair of 4-i groups): load u16 [128,4096], DVE
           tmp=u16*out_rep, halving-tree over d -> agreement a,
           b_state update, exp (ACT), Z (DVE reduce over n free dim),
           c=e/Z (ACT per-partition scale), c_rep half-depth doubling
           (ACT), sm=u16*c_rep in 2 d-chunks (DVE), ones-matmul
           accumulates s partial in psum (PE). AllReduce+squash between
           sweeps; final partial summed+squashed on host.

Layouts: u16 partition p = 32*j + b (j = i mod 4 within group), free
(d,n) d-major so d-reductions/broadcasts are contiguous halving trees.
"""
import sys
import types

sys.path.insert(0, "/opt/trn_rl_repo")

import numpy as np

from concourse import bacc, tile, mybir
from concourse.bass_utils import run_bass_kernel_spmd

f32 = mybir.dt.float32
f16 = mybir.dt.float16
AX = mybir.AxisListType
OP = mybir.AluOpType
AF = mybir.ActivationFunctionType

B, N, I, D, K = 32, 64, 2048, 32, 16
NCORES = 8
IL = I // NCORES          # 256 local input capsules
G = IL // 4               # 64 groups of 4 i
NP = G // 2               # 32 group-pairs
DN = D * N                # 2048 free elements per group, d-major
INV_LOG2 = float(1.0 / np.log(2.0))


def _install_ntff_hook():
    if "antenv.axon_hooks" in sys.modules:
        return
    try:
        mod = types.ModuleType("antenv.axon_hooks")
        state = {"hook": None}
        mod.set_axon_ntff_profile_hook = lambda h: state.__setitem__("hook", h)
        mod.get_axon_ntff_profile_hook = lambda: state["hook"]
        sys.modules["antenv.axon_hooks"] = mod
        import antenv
        antenv.axon_hooks = mod
        from trn_agent_boot.trn_boot import _ntff_profile_via_ctypes
        mod.set_axon_ntff_profile_hook(
            _ntff_profile_via_ctypes("/opt/axon/libaxon_pjrt.so"))
    except Exception:
        pass


def _build():
    nc = bacc.Bacc("TRN2", target_bir_lowering=False, debug=False,
                   num_devices=NCORES)

    w_t2 = nc.dram_tensor("w_t2", [NP, 128, DN], f16, kind="ExternalInput")
    x_bd = nc.dram_tensor("x_bd", [128, NP, 128], f16, kind="ExternalInput")
    s2_part = nc.dram_tensor("s2_part", [B, DN], f32, kind="ExternalOutput")

    u_store = nc.dram_tensor("u_store", [G, 128, DN], f16)
    cc_in = [nc.dram_tensor(f"cc_in{r}", [B, DN], f32) for r in range(2)]
    cc_out = [nc.dram_tensor(f"cc_out{r}", [B, DN], f32, addr_space="Shared")
              for r in range(2)]

    ones4_np = np.zeros((128, 32), np.float16)
    for p in range(128):
        ones4_np[p, p % 32] = 1.0
    ones4 = nc.inline_tensor(ones4_np, name="ones4")
    ones4f = nc.inline_tensor(ones4_np.astype(np.float32), name="ones4f")

    core_ids = list(range(NCORES))

    with tile.TileContext(nc) as tc:
        with tc.tile_pool(name="const", bufs=1) as constp, \
             tc.tile_pool(name="tail", bufs=1) as tail, \
             tc.tile_pool(name="small", bufs=4) as small, \
             tc.tile_pool(name="bstate", bufs=1) as bstate, \
             tc.tile_pool(name="psacc", bufs=1, space="PSUM") as psacc:

            ones_sb = constp.tile([128, 32], f16)
            nc.sync.dma_start(ones_sb[:], ones4[:])
            ones_sbf = constp.tile([128, 32], f32)
            nc.sync.dma_start(ones_sbf[:], ones4f[:])
            out_rep = [constp.tile([128, DN], f16, tag=f"orep{r}",
                                   name=f"orep{r}") for r in range(2)]

            def squash_to_outrep(s_sb, orep, pre_scale):
                """orep [128, (d,n)] f16 <- x4-replicated squash(s_sb*pre_scale)."""
                ps2 = float(pre_scale * pre_scale)
                sq = tail.tile([32, D, N], f32, tag="t_sq")
                nc.scalar.square(sq[:],
                                 s_sb[:].rearrange("p (d n) -> p d n", n=N))
                cur, d = sq, D
                while d > 2:
                    nxt = tail.tile([32, d // 2, N], f32, tag=f"t_tr{d}")
                    nc.vector.tensor_add(nxt[:], cur[:, 0:d // 2, :],
                                         cur[:, d // 2:d, :])
                    cur, d = nxt, d // 2
                sn = tail.tile([32, 1, N], f32, tag="t_sn")
                nc.vector.tensor_add(sn[:], cur[:, 0:1, :], cur[:, 1:2, :])
                r_ = tail.tile([32, N], f32, tag="t_r")
                nc.scalar.activation(r_[:], sn[:, 0, :], AF.Sqrt,
                                     bias=0.0, scale=ps2)
                den = tail.tile([32, N], f32, tag="t_den")
                nc.vector.tensor_scalar(den[:], sn[:, 0, :], ps2, 1.0,
                                        OP.mult, OP.add)
                rd = tail.tile([32, N], f32, tag="t_rd")
                nc.vector.reciprocal(rd[:], den[:])
                fac = tail.tile([32, N], f32, tag="t_fac")
                nc.vector.scalar_tensor_tensor(fac[:], r_[:],
                                               float(pre_scale), rd[:],
                                               op0=OP.mult, op1=OP.mult)
                frep = tail.tile([32, D, N], f32, tag="t_frep")
                nc.scalar.copy(frep[:, 0:1, :], fac[:].unsqueeze(1))
                d = 1
                while d < D:
                    nc.scalar.copy(frep[:, d:2 * d, :], frep[:, 0:d, :])
                    d *= 2
                o16 = tail.tile([32, D, N], f16, tag="t_o16")
                nc.vector.tensor_mul(
                    o16[:], s_sb[:].rearrange("p (d n) -> p d n", n=N), frep[:])
                for j in range(4):
                    nc.sync.dma_start(
                        orep[32 * j:32 * j + 32, :],
                        o16[:].rearrange("p d n -> p (d n)"))

            # ---------------- sweep 1: u_hat + S0 ----------------
            s0_ps = psacc.tile([B, DN], f32, tag="sacc")
            with tc.tile_pool(name="xw", bufs=1) as xw, \
                 tc.tile_pool(name="wp", bufs=3) as wp, \
                 tc.tile_pool(name="u16s1", bufs=3) as u16s1, \
                 tc.tile_pool(name="psum1", bufs=2, space="PSUM") as psum1:
                xbd_sb = xw.tile([128, NP, 128], f16)
                nc.sync.dma_start(xbd_sb[:], x_bd[:])
                s0_sb = xw.tile([128, DN], f32)
                for gp in range(NP):
                    wt = wp.tile([128, DN], f16)
                    nc.sync.dma_start(wt[:], w_t2[gp])
                    for gs in range(2):
                        g = 2 * gp + gs
                        u16 = u16s1.tile([128, DN], f16)
                        for h in range(2):
                            pu = psum1.tile([128, DN // 2], f32)
                            for ch in range(2):
                                nc.tensor.matmul(
                                    pu[:, 512 * ch:512 * (ch + 1)],
                                    lhsT=xbd_sb[64 * gs:64 * (gs + 1), gp, :],
                                    rhs=wt[64 * gs:64 * (gs + 1),
                                           1024 * h + 512 * ch:
                                           1024 * h + 512 * (ch + 1)],
                                    start=True, stop=True)
                            nc.scalar.copy(
                                u16[:, 1024 * h:1024 * (h + 1)], pu[:])
                            sl = s0_sb[:, 1024 * h:1024 * (h + 1)]
                            if gp == 0 and gs == 0:
                                nc.vector.tensor_copy(sl, pu[:])
                            else:
                                nc.vector.tensor_add(sl, sl, pu[:])
                        nc.sync.dma_start(u_store[g], u16[:])

                # fold j-slots: s0_ps[32, DN] = ones4f.T @ s0_sb
                for ch in range(4):
                    nc.tensor.matmul(
                        s0_ps[:, 512 * ch:512 * (ch + 1)],
                        lhsT=ones_sbf[:],
                        rhs=s0_sb[:, 512 * ch:512 * (ch + 1)],
                        start=True, stop=True)
                # S0 allreduce + squash -> out_rep[0]
                s0_dr = tail.tile([B, DN], f32, tag="t_drain")
                nc.scalar.copy(s0_dr[:], s0_ps[:])
                nc.sync.dma_start(cc_in[0][:], s0_dr[:])
                nc.gpsimd.collective_compute(
                    "AllReduce", OP.add, ins=[cc_in[0][:]],
                    outs=[cc_out[0][:]], replica_groups=[core_ids])
                s0_all = tail.tile([B, DN], f32, tag="t_all")
                nc.sync.dma_start(s0_all[:], cc_out[0][:])
                squash_to_outrep(s0_all, out_rep[0], 1.0 / 64.0)

            # ---------------- sweeps 2 and 3: routing ----------------
            with tc.tile_pool(name="u16p", bufs=4) as u16p, \
                 tc.tile_pool(name="crepp", bufs=4) as crepp, \
                 tc.tile_pool(name="big", bufs=4) as big, \
                 tc.tile_pool(name="tree", bufs=3) as tree:
                bs_tiles = []
                for it in range(2):
                    s_ps = psacc.tile([B, DN], f32, tag="sacc")
                    first_mm = True
                    for gp in range(NP):
                        u16 = u16p.tile([128, 2, DN], f16)
                        nc.sync.dma_start(
                            u16[:],
                            u_store[2 * gp:2 * gp + 2].transpose([1, 0, 2]))
                        u4 = u16[:].rearrange("p a (d n) -> p a d n", n=N)
                        orep4 = (out_rep[it][:]
                                 .rearrange("p (d n) -> p d n", n=N)
                                 .unsqueeze(1).broadcast_to([128, 2, D, N]))
                        tmp = big.tile([128, 2, D, N], f16, tag="tmp")
                        nc.vector.tensor_mul(tmp[:], u4, orep4)
                        cur, d = tmp, D
                        while d > 2:
                            nxt = tree.tile([128, 2, d // 2, N], f16,
                                            tag=f"tr{d}")
                            nc.vector.tensor_add(nxt[:], cur[:, :, 0:d // 2, :],
                                                 cur[:, :, d // 2:d, :])
                            cur, d = nxt, d // 2
                        if it == 0:
                            bs = bstate.tile([128, 2, 1, N], f32,
                                             tag=f"bs{gp}", name=f"bs{gp}")
                            bs_tiles.append(bs)
                            nc.vector.tensor_add(bs[:], cur[:, :, 0:1, :],
                                                 cur[:, :, 1:2, :])
                        else:
                            bs = bs_tiles[gp]
                            a2 = small.tile([128, 2, 1, N], f32, tag="a2")
                            nc.vector.tensor_add(a2[:], cur[:, :, 0:1, :],
                                                 cur[:, :, 1:2, :])
                            nc.vector.tensor_add(bs[:], bs[:], a2[:])
                        m2 = small.tile([128, 2, 1], f32, tag="m2")
                        nc.vector.tensor_reduce(out=m2[:], in_=bs[:, :, 0, :],
                                                axis=AX.X, op=OP.max)
                        nm2 = small.tile([128, 2, 1], f32, tag="nm2")
                        nc.vector.tensor_scalar_mul(nm2[:], m2[:], -INV_LOG2)
                        e2 = small.tile([128, 2, N], f16, tag="e2")
                        for gs in range(2):
                            nc.scalar.activation(e2[:, gs, :], bs[:, gs, 0, :],
                                                 AF.Exp, bias=nm2[:, gs, :],
                                                 scale=INV_LOG2)
                        z2 = small.tile([128, 2, 1], f32, tag="z2")
                        nc.vector.tensor_reduce(out=z2[:], in_=e2[:],
                                                axis=AX.X, op=OP.add)
                        rz = small.tile([128, 2, 1], f32, tag="rz")
                        nc.vector.reciprocal(rz[:], z2[:])
                        cz4 = small.tile([128, 2, 32], f16, tag="cz4")
                        for gs in range(2):
                            nc.vector.tensor_mul(
                                cz4[:, gs, :], ones_sb[:],
                                rz[:, gs, :].broadcast_to([128, 32]))
                        crep = crepp.tile([128, 2, D // 2, N], f16, tag="crep")
                        nc.scalar.copy(crep[:, :, 0:1, :], e2[:].unsqueeze(2))
                        d = 1
                        while d < D // 2:
                            nc.scalar.copy(crep[:, :, d:2 * d, :],
                                           crep[:, :, 0:d, :])
                            d *= 2
                        sm = big.tile([128, 2, D, N], f16, tag="sm")
                        for hd in range(2):
                            nc.vector.tensor_mul(
                                sm[:, :, 16 * hd:16 * (hd + 1), :],
                                u4[:, :, 16 * hd:16 * (hd + 1), :], crep[:])
                        smf = sm[:].rearrange("p a d n -> p a (d n)")
                        for gs in range(2):
                            for ch in range(4):
                                nc.tensor.matmul(
                                    s_ps[:, 512 * ch:512 * (ch + 1)],
                                    lhsT=cz4[:, gs, :],
                                    rhs=smf[:, gs, 512 * ch:512 * (ch + 1)],
                                    start=first_mm,
                                    stop=(gp == NP - 1 and gs == 1),
                                    skip_group_check=True)
                            first_mm = False

                    s_sb = tail.tile([B, DN], f32, tag="t_drain")
                    nc.scalar.copy(s_sb[:], s_ps[:])
                    if it == 0:
                        nc.sync.dma_start(cc_in[1][:], s_sb[:])
                        nc.gpsimd.collective_compute(
                            "AllReduce", OP.add, ins=[cc_in[1][:]],
                            outs=[cc_out[1][:]], replica_groups=[core_ids])
                        s_all = tail.tile([B, DN], f32, tag="t_all")
                        nc.sync.dma_start(s_all[:], cc_out[1][:])
                        squash_to_outrep(s_all, out_rep[1], 1.0)
                    else:
                        nc.sync.dma_start(s2_part[:], s_sb[:])

    nc.compile()
    return nc


_NC_CACHE = {}


def _get_nc():
    if "nc" not in _NC_CACHE:
        _NC_CACHE["nc"] = _build()
    return _NC_CACHE["nc"]


def _prep_core(x_c, w_c):
    """x_c [B, IL, K] f32, w_c [N, IL, D, K] f32 -> in_map dict."""
    wt = np.ascontiguousarray(w_c.transpose(1, 3, 2, 0))  # [IL, K, D, N]
    wt2 = wt.reshape(NP, 8, K, DN).reshape(NP, 128, DN).astype(np.float16)
    xt = x_c.transpose(1, 2, 0)  # [IL, K, B]
    x_bd = np.zeros((128, NP, 128), np.float16)
    for g in range(G):
        q, s = g // 2, g % 2
        for j in range(4):
            i = 4 * g + j
            x_bd[s * 64 + j * 16:s * 64 + j * 16 + K, q,
                 j * 32:j * 32 + 32] = xt[i].astype(np.float16)
    return {"w_t2": wt2, "x_bd": x_bd}


def _squash_np(v):
    sn = np.sum(v * v, axis=-1, keepdims=True)
    return np.sqrt(sn) / (1.0 + sn) * v


def _run(inputs, W, trace=False):
    _install_ntff_hook()
    nc = _get_nc()
    x = np.asarray(inputs, np.float32)
    Wf = np.asarray(W, np.float32)
    in_maps = []
    for c in range(NCORES):
        sl = slice(c * IL, (c + 1) * IL)
        in_maps.append(_prep_core(x[:, sl, :], Wf[:, sl, :, :]))
    res = run_bass_kernel_spmd(nc, in_maps, list(range(NCORES)), trace=trace)
    s2 = np.zeros((B, DN), np.float64)
    for c in range(NCORES):
        s2 += res.results[c]["s2_part"].astype(np.float64)
    s2 = s2.reshape(B, D, N).transpose(0, 2, 1).astype(np.float32)
    out = _squash_np(s2).astype(np.float32)
    return out, res


def kernel(inputs, W):
    out, _ = _run(inputs, W, trace=False)
    return out


# revision 16
# speedup vs baseline: 1.0560x; 1.0293x over previous
"""Capsule-routing kernel for 8 Trainium2 NeuronCores.

Problem: u_hat = einsum('nidk,bik->bnid', W, x); 3 rounds of dynamic
routing (softmax over n, weighted sum over i, squash, agreement update).

Sharding: input-capsule axis i (2048) split 8 ways -> 256 i per core.
Softmax over n is local; the per-iteration weighted sum s[b,n,d] is a
partial over local i, combined with an on-device AllReduce (iterations
1,2) or on the host (final iteration).

Per-core schedule (B=32, N=64, IL=256, D=32, K=16):
  sweep 1: u_hat via TensorE (block-diag x lhsT, K=64, M=128 -> psum
           [(j,b), (d,n)]), drain-cast to fp16, store to DRAM; S0
           accumulated in psum via ones-matmul. AllReduce S0, squash
           -> out0 [128, 2048] fp16 (partition-replicated x4).
  sweep 2/3 (per pair of 4-i groups): load u16 [128,4096], DVE
           tmp=u16*out_rep, halving-tree over d -> agreement a,
           b_state update, exp (ACT), Z (DVE reduce over n free dim),
           c=e/Z (ACT per-partition scale), c_rep half-depth doubling
           (ACT), sm=u16*c_rep in 2 d-chunks (DVE), ones-matmul
           accumulates s partial in psum (PE). AllReduce+squash between
           sweeps; final partial summed+squashed on host.

Layouts: u16 partition p = 32*j + b (j = i mod 4 within group), free
(d,n) d-major so d-reductions/broadcasts are contiguous halving trees.
"""
import sys
import types

sys.path.insert(0, "/opt/trn_rl_repo")

import numpy as np

from concourse import bacc, tile, mybir
from concourse.bass_utils import run_bass_kernel_spmd

f32 = mybir.dt.float32
f16 = mybir.dt.float16
AX = mybir.AxisListType
OP = mybir.AluOpType
AF = mybir.ActivationFunctionType

B, N, I, D, K = 32, 64, 2048, 32, 16
NCORES = 8
IL = I // NCORES          # 256 local input capsules
G = IL // 4               # 64 groups of 4 i
NP = G // 2               # 32 group-pairs
DN = D * N                # 2048 free elements per group, d-major
INV_LOG2 = float(1.0 / np.log(2.0))


def _install_ntff_hook():
    if "antenv.axon_hooks" in sys.modules:
        return
    try:
        mod = types.ModuleType("antenv.axon_hooks")
        state = {"hook": None}
        mod.set_axon_ntff_profile_hook = lambda h: state.__setitem__("hook", h)
        mod.get_axon_ntff_profile_hook = lambda: state["hook"]
        sys.modules["antenv.axon_hooks"] = mod
        import antenv
        antenv.axon_hooks = mod
        from trn_agent_boot.trn_boot import _ntff_profile_via_ctypes
        mod.set_axon_ntff_profile_hook(
            _ntff_profile_via_ctypes("/opt/axon/libaxon_pjrt.so"))
    except Exception:
        pass


def _build():
    nc = bacc.Bacc("TRN2", target_bir_lowering=False, debug=False,
                   num_devices=NCORES)

    w_t2 = nc.dram_tensor("w_t2", [NP, 128, DN], f16, kind="ExternalInput")
    x_bd = nc.dram_tensor("x_bd", [128, NP, 128], f16, kind="ExternalInput")
    s2_part = nc.dram_tensor("s2_part", [B, DN], f32, kind="ExternalOutput")

    u_store = nc.dram_tensor("u_store", [G, 128, DN], f16)
    cc_in = [nc.dram_tensor(f"cc_in{r}", [B, DN], f32) for r in range(2)]
    cc_out = [nc.dram_tensor(f"cc_out{r}", [B, DN], f32, addr_space="Shared")
              for r in range(2)]

    ones4_np = np.zeros((128, 32), np.float16)
    for p in range(128):
        ones4_np[p, p % 32] = 1.0
    ones4 = nc.inline_tensor(ones4_np, name="ones4")
    ones4f = nc.inline_tensor(ones4_np.astype(np.float32), name="ones4f")

    core_ids = list(range(NCORES))

    with tile.TileContext(nc) as tc:
        with tc.tile_pool(name="const", bufs=1) as constp, \
             tc.tile_pool(name="tail", bufs=1) as tail, \
             tc.tile_pool(name="small", bufs=4) as small, \
             tc.tile_pool(name="bstate", bufs=1) as bstate, \
             tc.tile_pool(name="psacc", bufs=1, space="PSUM") as psacc:

            ones_sb = constp.tile([128, 32], f16)
            nc.sync.dma_start(ones_sb[:], ones4[:])
            ones_sbf = constp.tile([128, 32], f32)
            nc.sync.dma_start(ones_sbf[:], ones4f[:])
            out_rep = [constp.tile([128, DN], f16, tag=f"orep{r}",
                                   name=f"orep{r}") for r in range(2)]

            def squash_to_outrep(s_sb, orep, pre_scale):
                """orep [128, (d,n)] f16 <- x4-replicated squash(s_sb*pre_scale)."""
                ps2 = float(pre_scale * pre_scale)
                sq = tail.tile([32, D, N], f32, tag="t_sq")
                nc.scalar.square(sq[:],
                                 s_sb[:].rearrange("p (d n) -> p d n", n=N))
                cur, d = sq, D
                while d > 2:
                    nxt = tail.tile([32, d // 2, N], f32, tag=f"t_tr{d}")
                    nc.vector.tensor_add(nxt[:], cur[:, 0:d // 2, :],
                                         cur[:, d // 2:d, :])
                    cur, d = nxt, d // 2
                sn = tail.tile([32, 1, N], f32, tag="t_sn")
                nc.vector.tensor_add(sn[:], cur[:, 0:1, :], cur[:, 1:2, :])
                r_ = tail.tile([32, N], f32, tag="t_r")
                nc.scalar.activation(r_[:], sn[:, 0, :], AF.Sqrt,
                                     bias=0.0, scale=ps2)
                den = tail.tile([32, N], f32, tag="t_den")
                nc.vector.tensor_scalar(den[:], sn[:, 0, :], ps2, 1.0,
                                        OP.mult, OP.add)
                rd = tail.tile([32, N], f32, tag="t_rd")
                nc.vector.reciprocal(rd[:], den[:])
                fac = tail.tile([32, N], f32, tag="t_fac")
                nc.vector.scalar_tensor_tensor(fac[:], r_[:],
                                               float(pre_scale), rd[:],
                                               op0=OP.mult, op1=OP.mult)
                frep = tail.tile([32, D, N], f32, tag="t_frep")
                nc.scalar.copy(frep[:, 0:1, :], fac[:].unsqueeze(1))
                d = 1
                while d < D:
                    nc.scalar.copy(frep[:, d:2 * d, :], frep[:, 0:d, :])
                    d *= 2
                o16 = tail.tile([32, D, N], f16, tag="t_o16")
                nc.vector.tensor_mul(
                    o16[:], s_sb[:].rearrange("p (d n) -> p d n", n=N), frep[:])
                for j in range(4):
                    nc.sync.dma_start(
                        orep[32 * j:32 * j + 32, :],
                        o16[:].rearrange("p d n -> p (d n)"))

            # ---------------- sweep 1: u_hat + S0 ----------------
            s0_ps = psacc.tile([B, DN], f32, tag="sacc")
            with tc.tile_pool(name="xw", bufs=1) as xw, \
                 tc.tile_pool(name="wp", bufs=3) as wp, \
                 tc.tile_pool(name="u16s1", bufs=3) as u16s1, \
                 tc.tile_pool(name="psum1", bufs=2, space="PSUM") as psum1:
                xbd_sb = xw.tile([128, NP, 128], f16)
                nc.sync.dma_start(xbd_sb[:], x_bd[:])
                subacc = [xw.tile([128, DN], f16, tag=f"sa{k}", name=f"sa{k}")
                          for k in range(8)]
                for gp in range(NP):
                    wt = wp.tile([128, DN], f16)
                    nc.sync.dma_start(wt[:], w_t2[gp])
                    for gs in range(2):
                        g = 2 * gp + gs
                        u16 = u16s1.tile([128, DN], f16)
                        for h in range(2):
                            pu = psum1.tile([128, DN // 2], f32)
                            for ch in range(2):
                                nc.tensor.matmul(
                                    pu[:, 512 * ch:512 * (ch + 1)],
                                    lhsT=xbd_sb[64 * gs:64 * (gs + 1), gp, :],
                                    rhs=wt[64 * gs:64 * (gs + 1),
                                           1024 * h + 512 * ch:
                                           1024 * h + 512 * (ch + 1)],
                                    start=True, stop=True)
                            nc.scalar.copy(
                                u16[:, 1024 * h:1024 * (h + 1)], pu[:])
                        sa = subacc[g // 8]
                        if g % 8 == 0:
                            nc.vector.tensor_copy(sa[:], u16[:])
                        else:
                            nc.vector.tensor_add(sa[:], sa[:], u16[:])
                        nc.sync.dma_start(u_store[g], u16[:])

                # merge sub-accumulators (f16), then fold j-slots via matmul
                for a, b_ in [(0, 1), (2, 3), (4, 5), (6, 7), (0, 2),
                              (4, 6), (0, 4)]:
                    nc.vector.tensor_add(subacc[a][:], subacc[a][:],
                                         subacc[b_][:])
                for ch in range(4):
                    nc.tensor.matmul(
                        s0_ps[:, 512 * ch:512 * (ch + 1)],
                        lhsT=ones_sb[:],
                        rhs=subacc[0][:, 512 * ch:512 * (ch + 1)],
                        start=True, stop=True)
                # S0 allreduce + squash -> out_rep[0]
                s0_dr = tail.tile([B, DN], f32, tag="t_drain")
                nc.scalar.copy(s0_dr[:], s0_ps[:])
                nc.sync.dma_start(cc_in[0][:], s0_dr[:])
                nc.gpsimd.collective_compute(
                    "AllReduce", OP.add, ins=[cc_in[0][:]],
                    outs=[cc_out[0][:]], replica_groups=[core_ids])
                s0_all = tail.tile([B, DN], f32, tag="t_all")
                nc.sync.dma_start(s0_all[:], cc_out[0][:])
                squash_to_outrep(s0_all, out_rep[0], 1.0 / 64.0)

            # ---------------- sweeps 2 and 3: routing ----------------
            with tc.tile_pool(name="u16p", bufs=4) as u16p, \
                 tc.tile_pool(name="crepp", bufs=4) as crepp, \
                 tc.tile_pool(name="big", bufs=4) as big, \
                 tc.tile_pool(name="tree", bufs=3) as tree:
                bs_tiles = []
                for it in range(2):
                    s_ps = psacc.tile([B, DN], f32, tag="sacc")
                    first_mm = True
                    for gp in range(NP):
                        u16 = u16p.tile([128, 2, DN], f16)
                        nc.sync.dma_start(
                            u16[:],
                            u_store[2 * gp:2 * gp + 2].transpose([1, 0, 2]))
                        u4 = u16[:].rearrange("p a (d n) -> p a d n", n=N)
                        orep4 = (out_rep[it][:]
                                 .rearrange("p (d n) -> p d n", n=N)
                                 .unsqueeze(1).broadcast_to([128, 2, D, N]))
                        tmp = big.tile([128, 2, D, N], f16, tag="tmp")
                        nc.vector.tensor_mul(tmp[:], u4, orep4)
                        cur, d = tmp, D
                        while d > 2:
                            nxt = tree.tile([128, 2, d // 2, N], f16,
                                            tag=f"tr{d}")
                            nc.vector.tensor_add(nxt[:], cur[:, :, 0:d // 2, :],
                                                 cur[:, :, d // 2:d, :])
                            cur, d = nxt, d // 2
                        if it == 0:
                            bs = bstate.tile([128, 2, 1, N], f32,
                                             tag=f"bs{gp}", name=f"bs{gp}")
                            bs_tiles.append(bs)
                            nc.vector.tensor_add(bs[:], cur[:, :, 0:1, :],
                                                 cur[:, :, 1:2, :])
                        else:
                            bs = bs_tiles[gp]
                            a2 = small.tile([128, 2, 1, N], f32, tag="a2")
                            nc.vector.tensor_add(a2[:], cur[:, :, 0:1, :],
                                                 cur[:, :, 1:2, :])
                            nc.vector.tensor_add(bs[:], bs[:], a2[:])
                        m2 = small.tile([128, 2, 1], f32, tag="m2")
                        nc.vector.tensor_reduce(out=m2[:], in_=bs[:, :, 0, :],
                                                axis=AX.X, op=OP.max)
                        nm2 = small.tile([128, 2, 1], f32, tag="nm2")
                        nc.vector.tensor_scalar_mul(nm2[:], m2[:], -INV_LOG2)
                        e2 = small.tile([128, 2, N], f16, tag="e2")
                        for gs in range(2):
                            nc.scalar.activation(e2[:, gs, :], bs[:, gs, 0, :],
                                                 AF.Exp, bias=nm2[:, gs, :],
                                                 scale=INV_LOG2)
                        z2 = small.tile([128, 2, 1], f32, tag="z2")
                        nc.vector.tensor_reduce(out=z2[:], in_=e2[:],
                                                axis=AX.X, op=OP.add)
                        rz = small.tile([128, 2, 1], f32, tag="rz")
                        nc.vector.reciprocal(rz[:], z2[:])
                        cz4 = small.tile([128, 2, 32], f16, tag="cz4")
                        for gs in range(2):
                            nc.vector.tensor_mul(
                                cz4[:, gs, :], ones_sb[:],
                                rz[:, gs, :].broadcast_to([128, 32]))
                        crep = crepp.tile([128, 2, D // 2, N], f16, tag="crep")
                        nc.scalar.copy(crep[:, :, 0:1, :], e2[:].unsqueeze(2))
                        d = 1
                        while d < D // 2:
                            nc.scalar.copy(crep[:, :, d:2 * d, :],
                                           crep[:, :, 0:d, :])
                            d *= 2
                        sm = big.tile([128, 2, D, N], f16, tag="sm")
                        for hd in range(2):
                            nc.vector.tensor_mul(
                                sm[:, :, 16 * hd:16 * (hd + 1), :],
                                u4[:, :, 16 * hd:16 * (hd + 1), :], crep[:])
                        smf = sm[:].rearrange("p a d n -> p a (d n)")
                        for gs in range(2):
                            for ch in range(4):
                                nc.tensor.matmul(
                                    s_ps[:, 512 * ch:512 * (ch + 1)],
                                    lhsT=cz4[:, gs, :],
                                    rhs=smf[:, gs, 512 * ch:512 * (ch + 1)],
                                    start=first_mm,
                                    stop=(gp == NP - 1 and gs == 1),
                                    skip_group_check=True)
                            first_mm = False

                    s_sb = tail.tile([B, DN], f32, tag="t_drain")
                    nc.scalar.copy(s_sb[:], s_ps[:])
                    if it == 0:
                        nc.sync.dma_start(cc_in[1][:], s_sb[:])
                        nc.gpsimd.collective_compute(
                            "AllReduce", OP.add, ins=[cc_in[1][:]],
                            outs=[cc_out[1][:]], replica_groups=[core_ids])
                        s_all = tail.tile([B, DN], f32, tag="t_all")
                        nc.sync.dma_start(s_all[:], cc_out[1][:])
                        squash_to_outrep(s_all, out_rep[1], 1.0)
                    else:
                        nc.sync.dma_start(s2_part[:], s_sb[:])

    nc.compile()
    return nc


_NC_CACHE = {}


def _get_nc():
    if "nc" not in _NC_CACHE:
        _NC_CACHE["nc"] = _build()
    return _NC_CACHE["nc"]


def _prep_core(x_c, w_c):
    """x_c [B, IL, K] f32, w_c [N, IL, D, K] f32 -> in_map dict."""
    wt = np.ascontiguousarray(w_c.transpose(1, 3, 2, 0))  # [IL, K, D, N]
    wt2 = wt.reshape(NP, 8, K, DN).reshape(NP, 128, DN).astype(np.float16)
    xt = x_c.transpose(1, 2, 0)  # [IL, K, B]
    x_bd = np.zeros((128, NP, 128), np.float16)
    for g in range(G):
        q, s = g // 2, g % 2
        for j in range(4):
            i = 4 * g + j
            x_bd[s * 64 + j * 16:s * 64 + j * 16 + K, q,
                 j * 32:j * 32 + 32] = xt[i].astype(np.float16)
    return {"w_t2": wt2, "x_bd": x_bd}


def _squash_np(v):
    sn = np.sum(v * v, axis=-1, keepdims=True)
    return np.sqrt(sn) / (1.0 + sn) * v


def _run(inputs, W, trace=False):
    _install_ntff_hook()
    nc = _get_nc()
    x = np.asarray(inputs, np.float32)
    Wf = np.asarray(W, np.float32)
    in_maps = []
    for c in range(NCORES):
        sl = slice(c * IL, (c + 1) * IL)
        in_maps.append(_prep_core(x[:, sl, :], Wf[:, sl, :, :]))
    res = run_bass_kernel_spmd(nc, in_maps, list(range(NCORES)), trace=trace)
    s2 = np.zeros((B, DN), np.float64)
    for c in range(NCORES):
        s2 += res.results[c]["s2_part"].astype(np.float64)
    s2 = s2.reshape(B, D, N).transpose(0, 2, 1).astype(np.float32)
    out = _squash_np(s2).astype(np.float32)
    return out, res


def kernel(inputs, W):
    out, _ = _run(inputs, W, trace=False)
    return out


# revision 17
# speedup vs baseline: 1.0826x; 1.0252x over previous
"""Capsule-routing kernel for 8 Trainium2 NeuronCores.

Problem: u_hat = einsum('nidk,bik->bnid', W, x); 3 rounds of dynamic
routing (softmax over n, weighted sum over i, squash, agreement update).

Sharding: input-capsule axis i (2048) split 8 ways -> 256 i per core.
Softmax over n is local; the per-iteration weighted sum s[b,n,d] is a
partial over local i, combined with an on-device AllReduce (iterations
1,2) or on the host (final iteration).

Per-core schedule (B=32, N=64, IL=256, D=32, K=16):
  sweep 1: u_hat via TensorE (block-diag x lhsT, K=64, M=128 -> psum
           [(j,b), (d,n)]), ACT drain-cast to fp16, store to DRAM; S0
           accumulated two-level in fp16 on DVE, j-slots folded with a
           ones-matmul. AllReduce S0, squash -> out0 [128, 2048] fp16
           (partition-replicated x4).
  sweep 2/3 (per pair of 4-i groups): load u16 [128,4096], DVE
           tmp=u16*out_rep, halving-tree over d -> agreement a,
           b_state update, exp (ACT), Z (DVE reduce over n free dim),
           max-subtracted exp to fp16 (ACT, bias=-max*INV_LOG2),
           e replicated over d by doubling copies (ACT), sm=u16*e_rep
           in 2 d-chunks (DVE); 1/Z folds into the accumulate-matmul's
           lhsT (cz4 = ones * 1/Z, per-partition), which sums the
           4 i-slots into s partial in psum (PE). AllReduce+squash
           between sweeps; final partial summed+squashed on host.

Layouts: u16 partition p = 32*j + b (j = i mod 4 within group), free
(d,n) d-major so d-reductions/broadcasts are contiguous halving trees.
"""
import sys
import types

sys.path.insert(0, "/opt/trn_rl_repo")

import numpy as np

from concourse import bacc, tile, mybir
from concourse.bass_utils import run_bass_kernel_spmd

f32 = mybir.dt.float32
f16 = mybir.dt.float16
AX = mybir.AxisListType
OP = mybir.AluOpType
AF = mybir.ActivationFunctionType

B, N, I, D, K = 32, 64, 2048, 32, 16
NCORES = 8
IL = I // NCORES          # 256 local input capsules
G = IL // 4               # 64 groups of 4 i
NP = G // 2               # 32 group-pairs
DN = D * N                # 2048 free elements per group, d-major
INV_LOG2 = float(1.0 / np.log(2.0))


def _install_ntff_hook():
    if "antenv.axon_hooks" in sys.modules:
        return
    try:
        mod = types.ModuleType("antenv.axon_hooks")
        state = {"hook": None}
        mod.set_axon_ntff_profile_hook = lambda h: state.__setitem__("hook", h)
        mod.get_axon_ntff_profile_hook = lambda: state["hook"]
        sys.modules["antenv.axon_hooks"] = mod
        import antenv
        antenv.axon_hooks = mod
        from trn_agent_boot.trn_boot import _ntff_profile_via_ctypes
        mod.set_axon_ntff_profile_hook(
            _ntff_profile_via_ctypes("/opt/axon/libaxon_pjrt.so"))
    except Exception:
        pass


def _build():
    nc = bacc.Bacc("TRN2", target_bir_lowering=False, debug=False,
                   num_devices=NCORES)

    w_t2 = nc.dram_tensor("w_t2", [NP, 128, DN], f16, kind="ExternalInput")
    x_bd = nc.dram_tensor("x_bd", [128, NP, 128], f16, kind="ExternalInput")
    s2_part = nc.dram_tensor("s2_part", [B, DN], f32, kind="ExternalOutput")

    u_store = nc.dram_tensor("u_store", [G, 128, DN], f16)
    cc_in = [nc.dram_tensor(f"cc_in{r}", [B, DN], f32) for r in range(2)]
    cc_out = [nc.dram_tensor(f"cc_out{r}", [B, DN], f32, addr_space="Shared")
              for r in range(2)]

    ones4_np = np.zeros((128, 32), np.float16)
    for p in range(128):
        ones4_np[p, p % 32] = 1.0
    ones4 = nc.inline_tensor(ones4_np, name="ones4")
    ones4f = nc.inline_tensor(ones4_np.astype(np.float32), name="ones4f")

    core_ids = list(range(NCORES))

    with tile.TileContext(nc) as tc:
        with tc.tile_pool(name="const", bufs=1) as constp, \
             tc.tile_pool(name="tail", bufs=1) as tail, \
             tc.tile_pool(name="small", bufs=4) as small, \
             tc.tile_pool(name="bstate", bufs=1) as bstate, \
             tc.tile_pool(name="psacc", bufs=1, space="PSUM") as psacc:

            ones_sb = constp.tile([128, 32], f16)
            nc.sync.dma_start(ones_sb[:], ones4[:])
            ones_sbf = constp.tile([128, 32], f32)
            nc.sync.dma_start(ones_sbf[:], ones4f[:])
            out_rep = [constp.tile([128, DN], f16, tag=f"orep{r}",
                                   name=f"orep{r}") for r in range(2)]

            def squash_to_outrep(s_sb, orep, pre_scale):
                """orep [128, (d,n)] f16 <- x4-replicated squash(s_sb*pre_scale)."""
                ps2 = float(pre_scale * pre_scale)
                sq = tail.tile([32, D, N], f32, tag="t_sq")
                nc.scalar.square(sq[:],
                                 s_sb[:].rearrange("p (d n) -> p d n", n=N))
                cur, d = sq, D
                while d > 2:
                    nxt = tail.tile([32, d // 2, N], f32, tag=f"t_tr{d}")
                    nc.vector.tensor_add(nxt[:], cur[:, 0:d // 2, :],
                                         cur[:, d // 2:d, :])
                    cur, d = nxt, d // 2
                sn = tail.tile([32, 1, N], f32, tag="t_sn")
                nc.vector.tensor_add(sn[:], cur[:, 0:1, :], cur[:, 1:2, :])
                r_ = tail.tile([32, N], f32, tag="t_r")
                nc.scalar.activation(r_[:], sn[:, 0, :], AF.Sqrt,
                                     bias=0.0, scale=ps2)
                den = tail.tile([32, N], f32, tag="t_den")
                nc.vector.tensor_scalar(den[:], sn[:, 0, :], ps2, 1.0,
                                        OP.mult, OP.add)
                rd = tail.tile([32, N], f32, tag="t_rd")
                nc.vector.reciprocal(rd[:], den[:])
                fac = tail.tile([32, N], f32, tag="t_fac")
                nc.vector.scalar_tensor_tensor(fac[:], r_[:],
                                               float(pre_scale), rd[:],
                                               op0=OP.mult, op1=OP.mult)
                frep = tail.tile([32, D, N], f32, tag="t_frep")
                nc.scalar.copy(frep[:, 0:1, :], fac[:].unsqueeze(1))
                d = 1
                while d < D:
                    nc.scalar.copy(frep[:, d:2 * d, :], frep[:, 0:d, :])
                    d *= 2
                o16 = tail.tile([32, D, N], f16, tag="t_o16")
                nc.vector.tensor_mul(
                    o16[:], s_sb[:].rearrange("p (d n) -> p d n", n=N), frep[:])
                for j in range(4):
                    nc.sync.dma_start(
                        orep[32 * j:32 * j + 32, :],
                        o16[:].rearrange("p d n -> p (d n)"))

            # ---------------- sweep 1: u_hat + S0 ----------------
            s0_ps = psacc.tile([B, DN], f32, tag="sacc")
            with tc.tile_pool(name="xw", bufs=1) as xw, \
                 tc.tile_pool(name="wp", bufs=3) as wp, \
                 tc.tile_pool(name="u16s1", bufs=3) as u16s1, \
                 tc.tile_pool(name="psum1", bufs=2, space="PSUM") as psum1:
                xbd_sb = xw.tile([128, NP, 128], f16)
                nc.sync.dma_start(xbd_sb[:], x_bd[:])
                subacc = [xw.tile([128, DN], f16, tag=f"sa{k}", name=f"sa{k}")
                          for k in range(8)]
                for gp in range(NP):
                    wt = wp.tile([128, DN], f16)
                    nc.sync.dma_start(wt[:], w_t2[gp])
                    for gs in range(2):
                        g = 2 * gp + gs
                        u16 = u16s1.tile([128, DN], f16)
                        for h in range(2):
                            pu = psum1.tile([128, DN // 2], f32)
                            for ch in range(2):
                                nc.tensor.matmul(
                                    pu[:, 512 * ch:512 * (ch + 1)],
                                    lhsT=xbd_sb[64 * gs:64 * (gs + 1), gp, :],
                                    rhs=wt[64 * gs:64 * (gs + 1),
                                           1024 * h + 512 * ch:
                                           1024 * h + 512 * (ch + 1)],
                                    start=True, stop=True)
                            nc.scalar.copy(
                                u16[:, 1024 * h:1024 * (h + 1)], pu[:])
                        sa = subacc[g // 8]
                        if g % 8 == 0:
                            nc.vector.tensor_copy(sa[:], u16[:])
                        else:
                            nc.vector.tensor_add(sa[:], sa[:], u16[:])
                        nc.sync.dma_start(u_store[g], u16[:])

                # merge sub-accumulators (f16), then fold j-slots via matmul
                for a, b_ in [(0, 1), (2, 3), (4, 5), (6, 7), (0, 2),
                              (4, 6), (0, 4)]:
                    nc.vector.tensor_add(subacc[a][:], subacc[a][:],
                                         subacc[b_][:])
                for ch in range(4):
                    nc.tensor.matmul(
                        s0_ps[:, 512 * ch:512 * (ch + 1)],
                        lhsT=ones_sb[:],
                        rhs=subacc[0][:, 512 * ch:512 * (ch + 1)],
                        start=True, stop=True)
                # S0 allreduce + squash -> out_rep[0]
                s0_dr = tail.tile([B, DN], f32, tag="t_drain")
                nc.scalar.copy(s0_dr[:], s0_ps[:])
                nc.sync.dma_start(cc_in[0][:], s0_dr[:])
                nc.gpsimd.collective_compute(
                    "AllReduce", OP.add, ins=[cc_in[0][:]],
                    outs=[cc_out[0][:]], replica_groups=[core_ids])
                s0_all = tail.tile([B, DN], f32, tag="t_all")
                nc.sync.dma_start(s0_all[:], cc_out[0][:])
                squash_to_outrep(s0_all, out_rep[0], 1.0 / 64.0)

            # ---------------- sweeps 2 and 3: routing ----------------
            with tc.tile_pool(name="u16p", bufs=4) as u16p, \
                 tc.tile_pool(name="crepp", bufs=4) as crepp, \
                 tc.tile_pool(name="big", bufs=4) as big, \
                 tc.tile_pool(name="tree", bufs=3) as tree:
                bs_tiles = []
                for it in range(2):
                    s_ps = psacc.tile([B, DN], f32, tag="sacc")
                    first_mm = True
                    for gp in range(NP):
                        u16 = u16p.tile([128, 2, DN], f16)
                        nc.sync.dma_start(
                            u16[:],
                            u_store[2 * gp:2 * gp + 2].transpose([1, 0, 2]))
                        u4 = u16[:].rearrange("p a (d n) -> p a d n", n=N)
                        orep4 = (out_rep[it][:]
                                 .rearrange("p (d n) -> p d n", n=N)
                                 .unsqueeze(1).broadcast_to([128, 2, D, N]))
                        tmp = big.tile([128, 2, D, N], f16, tag="tmp")
                        nc.vector.tensor_mul(tmp[:], u4, orep4)
                        cur, d = tmp, D
                        while d > 2:
                            nxt = tree.tile([128, 2, d // 2, N], f16,
                                            tag=f"tr{d}")
                            nc.vector.tensor_add(nxt[:], cur[:, :, 0:d // 2, :],
                                                 cur[:, :, d // 2:d, :])
                            cur, d = nxt, d // 2
                        if it == 0:
                            bs = bstate.tile([128, 2, 1, N], f32,
                                             tag=f"bs{gp}", name=f"bs{gp}")
                            bs_tiles.append(bs)
                            nc.vector.tensor_add(bs[:], cur[:, :, 0:1, :],
                                                 cur[:, :, 1:2, :])
                        else:
                            bs = bs_tiles[gp]
                            a2 = small.tile([128, 2, 1, N], f32, tag="a2")
                            nc.vector.tensor_add(a2[:], cur[:, :, 0:1, :],
                                                 cur[:, :, 1:2, :])
                            nc.vector.tensor_add(bs[:], bs[:], a2[:])
                        m2 = small.tile([128, 2, 1], f32, tag="m2")
                        nc.vector.tensor_reduce(out=m2[:], in_=bs[:, :, 0, :],
                                                axis=AX.X, op=OP.max)
                        nm2 = small.tile([128, 2, 1], f32, tag="nm2")
                        nc.vector.tensor_scalar_mul(nm2[:], m2[:], -INV_LOG2)
                        e2 = small.tile([128, 2, N], f16, tag="e2")
                        for gs in range(2):
                            nc.scalar.activation(e2[:, gs, :], bs[:, gs, 0, :],
                                                 AF.Exp, bias=nm2[:, gs, :],
                                                 scale=INV_LOG2)
                        z2 = small.tile([128, 2, 1], f32, tag="z2")
                        nc.vector.tensor_reduce(out=z2[:], in_=e2[:],
                                                axis=AX.X, op=OP.add)
                        rz = small.tile([128, 2, 1], f32, tag="rz")
                        nc.vector.reciprocal(rz[:], z2[:])
                        cz4 = small.tile([128, 2, 32], f16, tag="cz4")
                        for gs in range(2):
                            nc.vector.tensor_mul(
                                cz4[:, gs, :], ones_sb[:],
                                rz[:, gs, :].broadcast_to([128, 32]))
                        crep = crepp.tile([128, 2, D // 2, N], f16, tag="crep")
                        nc.scalar.copy(crep[:, :, 0:1, :], e2[:].unsqueeze(2))
                        d = 1
                        while d < D // 2:
                            nc.scalar.copy(crep[:, :, d:2 * d, :],
                                           crep[:, :, 0:d, :])
                            d *= 2
                        sm = big.tile([128, 2, D, N], f16, tag="sm")
                        for hd in range(2):
                            nc.vector.tensor_mul(
                                sm[:, :, 16 * hd:16 * (hd + 1), :],
                                u4[:, :, 16 * hd:16 * (hd + 1), :], crep[:])
                        smf = sm[:].rearrange("p a d n -> p a (d n)")
                        for gs in range(2):
                            for ch in range(4):
                                nc.tensor.matmul(
                                    s_ps[:, 512 * ch:512 * (ch + 1)],
                                    lhsT=cz4[:, gs, :],
                                    rhs=smf[:, gs, 512 * ch:512 * (ch + 1)],
                                    start=first_mm,
                                    stop=(gp == NP - 1 and gs == 1),
                                    skip_group_check=True)
                            first_mm = False

                    s_sb = tail.tile([B, DN], f32, tag="t_drain")
                    nc.scalar.copy(s_sb[:], s_ps[:])
                    if it == 0:
                        nc.sync.dma_start(cc_in[1][:], s_sb[:])
                        nc.gpsimd.collective_compute(
                            "AllReduce", OP.add, ins=[cc_in[1][:]],
                            outs=[cc_out[1][:]], replica_groups=[core_ids])
                        s_all = tail.tile([B, DN], f32, tag="t_all")
                        nc.sync.dma_start(s_all[:], cc_out[1][:])
                        squash_to_outrep(s_all, out_rep[1], 1.0)
                    else:
                        nc.sync.dma_start(s2_part[:], s_sb[:])

    nc.compile()
    return nc


_NC_CACHE = {}


def _get_nc():
    if "nc" not in _NC_CACHE:
        _NC_CACHE["nc"] = _build()
    return _NC_CACHE["nc"]


def _prep_core(x_c, w_c):
    """x_c [B, IL, K] f32, w_c [N, IL, D, K] f32 -> in_map dict."""
    wt = np.ascontiguousarray(w_c.transpose(1, 3, 2, 0))  # [IL, K, D, N]
    wt2 = wt.reshape(NP, 8, K, DN).reshape(NP, 128, DN).astype(np.float16)
    xt = x_c.transpose(1, 2, 0)  # [IL, K, B]
    x_bd = np.zeros((128, NP, 128), np.float16)
    for g in range(G):
        q, s = g // 2, g % 2
        for j in range(4):
            i = 4 * g + j
            x_bd[s * 64 + j * 16:s * 64 + j * 16 + K, q,
                 j * 32:j * 32 + 32] = xt[i].astype(np.float16)
    return {"w_t2": wt2, "x_bd": x_bd}


def _squash_np(v):
    sn = np.sum(v * v, axis=-1, keepdims=True)
    return np.sqrt(sn) / (1.0 + sn) * v


def _run(inputs, W, trace=False):
    _install_ntff_hook()
    nc = _get_nc()
    x = np.asarray(inputs, np.float32)
    Wf = np.asarray(W, np.float32)
    in_maps = []
    for c in range(NCORES):
        sl = slice(c * IL, (c + 1) * IL)
        in_maps.append(_prep_core(x[:, sl, :], Wf[:, sl, :, :]))
    res = run_bass_kernel_spmd(nc, in_maps, list(range(NCORES)), trace=trace)
    s2 = np.zeros((B, DN), np.float64)
    for c in range(NCORES):
        s2 += res.results[c]["s2_part"].astype(np.float64)
    s2 = s2.reshape(B, D, N).transpose(0, 2, 1).astype(np.float32)
    out = _squash_np(s2).astype(np.float32)
    return out, res


def kernel(inputs, W):
    out, _ = _run(inputs, W, trace=False)
    return out
